# revision 6
# baseline (speedup 1.0000x reference)
"""Trainium2 Bass kernel for nn_RegLoss (segment-reduce weighted regression loss).

Math: with per-class means m_c = S_c / max(n_c, 1), S_c = sum_{i: t_i=c} x_i,
    loss = sum_i w_i * ||x_i - m_{t_i}||^2 / sum_i w_i
         = (A - 2*sum_c m_c.T_c + sum_c W_c*||m_c||^2) / sum_i w_i
with A = sum_i w_i ||x_i||^2, T_c = sum_{i in c} w_i x_i, W_c = sum_{i in c} w_i.
Everything reduces to per-class segment sums + one global weighted square sum.

Sharding: rows are bucketed by class range (32 classes per bucket, 4 buckets
per core -> core k owns classes [128k, 128k+128)), padded to a fixed per-bucket
capacity. Per 128-row block the device builds a [128,32] one-hot from the local
class index and accumulates, via TensorE matmuls into PSUM:
  out12[d, c|c+32] += xsw_block.T @ [oh*rs | oh*sw]   -> S_c, T_c
  out3 [0:2, c]    += [v*sw, w*sw].T @ (oh*rs)        -> n_c, W_c
where xsw = sqrt(w)*x is the (host-prescaled) data, rs = 1/sqrt(w).  The
prescaling makes A = sum(xsw^2) an unweighted square-sum, done on ScalarE
(Square activation with accum_out).  Host combines the per-core partials.
"""

import contextlib
import sys

for _p in ("/opt/trn_rl_repo",):
    if _p not in sys.path:
        sys.path.insert(0, _p)

import numpy as np
import ml_dtypes

BF16 = ml_dtypes.bfloat16

# Problem constants (hardcoded per contract)
N = 500000
D = 128
C = 1000
NCORES = 8
BW = 16                 # classes per bucket
NBUCK = 8               # buckets per core
CSLOTS = NCORES * NBUCK * BW  # 1024 padded class slots
CAP = 8320              # padded rows per bucket (max observed 8172)
NBLK = CAP // 128       # blocks per bucket = 65
TOT = NBUCK * NBLK      # blocks per core = 520
SB = 40                 # blocks per supertile
NST = TOT // SB         # supertiles per core = 13

_CACHED_NC = None


def _emit_body(nc, mybir, xt, tcols_t, rs_t, sw_t, vwu_t, iota_t, stats_t,
               st_ps, nw_ps, sq_scr, sq_scr2, xp, ohp):
    AOp = mybir.AluOpType
    AF = mybir.ActivationFunctionType
    dtb = mybir.dt.bfloat16
    for s in range(NST):
        g0 = s * SB
        x_t = xp.tile([128, SB * 128], dtb, name="x_t", tag="x")
        nc.sync.dma_start(x_t[:], xt[:, g0 * 128 : (g0 + SB) * 128])

        oh_t = ohp.tile([128, SB * BW], dtb, name="oh_t", tag="oh")
        ohb_t = ohp.tile([128, SB * 2 * BW], dtb, name="ohb_t", tag="ohb")

        oh3 = oh_t[:].rearrange("p (j c) -> p j c", c=BW)
        i3 = iota_t[:].unsqueeze(1).broadcast_to((128, SB, BW))
        t3 = tcols_t[:, g0 : g0 + SB].unsqueeze(2).broadcast_to((128, SB, BW))
        nc.vector.tensor_tensor(oh3, i3, t3, AOp.is_equal)

        ohb3 = ohb_t[:].rearrange("p (j c) -> p j c", c=2 * BW)
        rs3 = rs_t[:, g0 : g0 + SB].unsqueeze(2).broadcast_to((128, SB, BW))
        sw3 = sw_t[:, g0 : g0 + SB].unsqueeze(2).broadcast_to((128, SB, BW))
        nc.vector.tensor_tensor(ohb3[:, :, 0:BW], oh3, rs3, AOp.mult)
        nc.vector.tensor_tensor(ohb3[:, :, BW : 2 * BW], oh3, sw3, AOp.mult)

        if s % 4 != 3:
            nc.scalar.activation(
                sq_scr[:], x_t[:], AF.Square, accum_out=stats_t[:, s : s + 1]
            )
        else:
            # offload 2 of 16 square-accums to the vector engine
            nc.vector.scalar_tensor_tensor(
                sq_scr2[:], x_t[:], 1.0, x_t[:], AOp.mult, AOp.mult,
                accum_out=stats_t[:, s : s + 1],
            )

        for j in range(SB):
            g = g0 + j
            b = g // NBLK
            lb = g % NBLK
            w2 = 2 * BW
            nc.tensor.matmul(
                st_ps[b // 2][:, (b % 2) * w2 : (b % 2) * w2 + w2],
                x_t[:, j * 128 : (j + 1) * 128],
                ohb_t[:, j * w2 : (j + 1) * w2],
                start=(lb == 0),
                stop=(lb == NBLK - 1),
            )
            nc.tensor.matmul(
                nw_ps[:, b * BW : (b + 1) * BW],
                vwu_t[:, g * 2 : g * 2 + 2],
                ohb_t[:, j * w2 : j * w2 + BW],
                start=(lb == 0),
                stop=(lb == NBLK - 1),
            )


def _build_nc(loop_reps=None):
    import concourse.mybir as mybir
    import concourse.tile as tile
    from concourse import bacc

    dtb = mybir.dt.bfloat16
    dtf = mybir.dt.float32
    nc = bacc.Bacc(None, target_bir_lowering=False, debug=False)

    xt = nc.dram_tensor("xt", [128, TOT * 128], dtb, kind="ExternalInput")
    tcol = nc.dram_tensor("tcols", [128, TOT], dtb, kind="ExternalInput")
    rscol = nc.dram_tensor("rscols", [128, TOT], dtb, kind="ExternalInput")
    swcol = nc.dram_tensor("swcols", [128, TOT], dtb, kind="ExternalInput")
    vwu = nc.dram_tensor("vwu", [128, TOT * 2], dtb, kind="ExternalInput")
    iota = nc.dram_tensor("iota", [128, BW], dtb, kind="ExternalInput")
    o_st = nc.dram_tensor("o_st", [128, NBUCK * 2 * BW], dtf, kind="ExternalOutput")
    o_nw = nc.dram_tensor("o_nw", [2, NBUCK * BW], dtf, kind="ExternalOutput")
    o_stats = nc.dram_tensor("o_stats", [128, NST], dtf, kind="ExternalOutput")

    with tile.TileContext(nc) as tc:
        with (
            tc.tile_pool(name="const", bufs=1) as constp,
            tc.tile_pool(name="xp", bufs=3) as xp,
            tc.tile_pool(name="ohp", bufs=3) as ohp,
            tc.tile_pool(name="scr", bufs=1) as scrp,
            tc.tile_pool(name="psum", bufs=1, space="PSUM") as pp,
            tc.tile_pool(name="outp", bufs=1) as outp,
        ):
            tcols_t = constp.tile([128, TOT], dtb, tag="tcols")
            nc.sync.dma_start(tcols_t[:], tcol[:])
            rs_t = constp.tile([128, TOT], dtb, tag="rs")
            nc.sync.dma_start(rs_t[:], rscol[:])
            sw_t = constp.tile([128, TOT], dtb, tag="sw")
            nc.sync.dma_start(sw_t[:], swcol[:])
            vwu_t = constp.tile([128, TOT * 2], dtb, tag="vwu")
            nc.sync.dma_start(vwu_t[:], vwu[:])
            iota_t = constp.tile([128, BW], dtb, tag="iota")
            nc.sync.dma_start(iota_t[:], iota[:])
            stats_t = constp.tile([128, NST], dtf, tag="stats")

            st_ps = [
                pp.tile([128, 4 * BW], dtf, name=f"st{m}", tag=f"st{m}")
                for m in range(NBUCK // 2)
            ]
            nw_ps = pp.tile([2, NBUCK * BW], dtf, tag="nw")

            sq_scr = scrp.tile([128, SB * 128], dtb, tag="sq")
            sq_scr2 = scrp.tile([128, SB * 128], dtb, tag="sq2")

            loop_cm = (
                tc.For_i(0, loop_reps, 1)
                if loop_reps is not None
                else contextlib.nullcontext()
            )
            with loop_cm:
                _emit_body(nc, mybir, xt, tcols_t, rs_t, sw_t, vwu_t, iota_t,
                           stats_t, st_ps, nw_ps, sq_scr, sq_scr2, xp, ohp)

            st_out = outp.tile([128, NBUCK * 2 * BW], dtf, tag="st_out")
            for m in range(NBUCK // 2):
                nc.vector.tensor_copy(
                    st_out[:, m * 4 * BW : (m + 1) * 4 * BW], st_ps[m][:]
                )
            nc.sync.dma_start(o_st[:], st_out[:])
            nw_out = outp.tile([2, NBUCK * BW], dtf, tag="nw_out")
            nc.vector.tensor_copy(nw_out[:], nw_ps[:])
            nc.sync.dma_start(o_nw[:], nw_out[:])
            nc.sync.dma_start(o_stats[:], stats_t[:])

    nc.finalize()
    return nc


def _get_nc():
    global _CACHED_NC
    if _CACHED_NC is None:
        _CACHED_NC = _build_nc()
    return _CACHED_NC


def _prepare_inputs(x, t, w):
    """Bucket rows by class range, pad, prescale, transpose to device layout."""
    sw = np.sqrt(np.maximum(w, 1e-24), dtype=np.float32)
    rs = (1.0 / sw).astype(np.float32)

    gb = t // BW  # global bucket 0..31
    order = np.argsort(gb, kind="stable")
    counts = np.bincount(gb, minlength=NCORES * NBUCK)
    if counts.max() > CAP:
        raise RuntimeError(f"bucket overflow: {counts.max()} > {CAP}")

    GB = NCORES * NBUCK
    xs = x[order] * sw[order, None]  # f32 [N, D]
    ts = (t[order] % BW).astype(np.float32)
    sws = sw[order]
    rss = rs[order]
    ws = w[order]

    Xp = np.zeros((GB, CAP, D), dtype=BF16)
    Tp = np.zeros((GB, CAP), dtype=BF16)
    Sp = np.zeros((GB, CAP), dtype=BF16)
    Rp = np.zeros((GB, CAP), dtype=BF16)
    Vp = np.zeros((GB, CAP, 2), dtype=BF16)
    off = 0
    for g in range(GB):
        cnt = int(counts[g])
        seg = slice(off, off + cnt)
        Xp[g, :cnt] = xs[seg].astype(BF16)
        Tp[g, :cnt] = ts[seg].astype(BF16)
        Sp[g, :cnt] = sws[seg].astype(BF16)
        Rp[g, :cnt] = rss[seg].astype(BF16)
        Vp[g, :cnt, 0] = sws[seg].astype(BF16)  # v * sw (v=1 for real rows)
        Vp[g, :cnt, 1] = (ws[seg] * sws[seg]).astype(BF16)  # w * sw
        off += cnt

    iota_arr = np.tile(np.arange(BW, dtype=np.float32), (128, 1)).astype(BF16)

    in_maps = []
    for k in range(NCORES):
        sl = slice(NBUCK * k, NBUCK * (k + 1))
        xt_k = np.ascontiguousarray(
            Xp[sl].reshape(TOT, 128, D).transpose(1, 0, 2).reshape(128, TOT * D)
        )
        tc_k = np.ascontiguousarray(Tp[sl].reshape(TOT, 128).T)
        sw_k = np.ascontiguousarray(Sp[sl].reshape(TOT, 128).T)
        rs_k = np.ascontiguousarray(Rp[sl].reshape(TOT, 128).T)
        vw_k = np.ascontiguousarray(
            Vp[sl].reshape(TOT, 128, 2).transpose(1, 0, 2).reshape(128, TOT * 2)
        )
        in_maps.append(
            {
                "xt": xt_k,
                "tcols": tc_k,
                "rscols": rs_k,
                "swcols": sw_k,
                "vwu": vw_k,
                "iota": iota_arr,
            }
        )
    return in_maps


def _combine(results):
    S = np.zeros((CSLOTS, D), dtype=np.float64)
    T = np.zeros((CSLOTS, D), dtype=np.float64)
    n = np.zeros(CSLOTS, dtype=np.float64)
    W = np.zeros(CSLOTS, dtype=np.float64)
    A = 0.0
    for k in range(NCORES):
        r = results[k]
        ost = np.asarray(r["o_st"], dtype=np.float64)
        onw = np.asarray(r["o_nw"], dtype=np.float64)
        A += float(np.asarray(r["o_stats"], dtype=np.float64).sum())
        for b in range(NBUCK):
            c0 = 128 * k + BW * b
            w2 = 2 * BW
            S[c0 : c0 + BW] = ost[:, w2 * b : w2 * b + BW].T
            T[c0 : c0 + BW] = ost[:, w2 * b + BW : w2 * b + w2].T
            n[c0 : c0 + BW] = onw[0, BW * b : BW * (b + 1)]
            W[c0 : c0 + BW] = onw[1, BW * b : BW * (b + 1)]

    n_int = np.round(n)
    means = S / np.maximum(n_int, 1.0)[:, None]
    Wsum = W.sum()
    total = A - 2.0 * float((means * T).sum()) + float(
        (W * (means * means).sum(axis=1)).sum()
    )
    return np.float32(total / Wsum)


def kernel(inputs, targets, weights, num_classes):
    from concourse.bass_utils import run_bass_kernel_spmd

    x = np.asarray(inputs, dtype=np.float32)
    t = np.asarray(targets).astype(np.int64)
    w = np.asarray(weights, dtype=np.float32)
    assert int(num_classes) == C, f"compiled for {C} classes, got {num_classes}"
    assert x.shape == (N, D) and t.shape == (N,) and w.shape == (N,)

    in_maps = _prepare_inputs(x, t, w)
    nc = _get_nc()
    res = run_bass_kernel_spmd(nc, in_maps, list(range(NCORES)))
    return _combine(res.results)


if __name__ == "__main__":
    rng = np.random.default_rng(0)
    x = rng.standard_normal((N, D)).astype(np.float32)
    t = rng.integers(0, C, N).astype(np.int64)
    w = rng.random(N).astype(np.float32)
    out = kernel(x, t, w, C)
    print("kernel output:", out)


# revision 8
# speedup vs baseline: 1.1064x; 1.1064x over previous
"""Trainium2 Bass kernel for nn_RegLoss (segment-reduce weighted regression loss).

Math: with per-class means m_c = S_c / max(n_c, 1), S_c = sum_{i: t_i=c} x_i,
    loss = sum_i w_i * ||x_i - m_{t_i}||^2 / sum_i w_i
         = (A - 2*sum_c m_c.T_c + sum_c W_c*||m_c||^2) / sum_i w_i
with A = sum_i w_i ||x_i||^2, T_c = sum_{i in c} w_i x_i, W_c = sum_{i in c} w_i.
Everything reduces to per-class segment sums + one global weighted square sum.

Sharding: rows are bucketed by class range (32 classes per bucket, 4 buckets
per core -> core k owns classes [128k, 128k+128)), padded to a fixed per-bucket
capacity. Per 128-row block the device builds a [128,32] one-hot from the local
class index and accumulates, via TensorE matmuls into PSUM:
  out12[d, c|c+32] += xsw_block.T @ [oh*rs | oh*sw]   -> S_c, T_c
  out3 [0:2, c]    += [v*sw, w*sw].T @ (oh*rs)        -> n_c, W_c
where xsw = sqrt(w)*x is the (host-prescaled) data, rs = 1/sqrt(w).  The
prescaling makes A = sum(xsw^2) an unweighted square-sum, done on ScalarE
(Square activation with accum_out).  Host combines the per-core partials.
"""

import contextlib
import sys

for _p in ("/opt/trn_rl_repo",):
    if _p not in sys.path:
        sys.path.insert(0, _p)

import numpy as np
import ml_dtypes

BF16 = ml_dtypes.bfloat16

# Problem constants (hardcoded per contract)
N = 500000
D = 128
C = 1000
NCORES = 8
BW = 16                 # classes per bucket
NBUCK = 8               # buckets per core
CSLOTS = NCORES * NBUCK * BW  # 1024 padded class slots
CAP = 8320              # padded rows per bucket (max observed 8172)
NBLK = CAP // 128       # blocks per bucket = 65
TOT = NBUCK * NBLK      # blocks per core = 520
SB = 40                 # blocks per supertile
NST = TOT // SB         # supertiles per core = 13

_CACHED_NC = None


def _emit_body(nc, mybir, xt, tcols_t, rs_t, sw_t, iota_t, stats_t,
               st_ps, sq_scr3, sq_scr, sq_scr2, xp, ohp):
    AOp = mybir.AluOpType
    AF = mybir.ActivationFunctionType
    dtb = mybir.dt.bfloat16
    RW = 130  # per-block rhs width: 128 x cols + vsw + wsw
    for s in range(NST):
        g0 = s * SB
        x_t = xp.tile([128, SB * RW], dtb, name="x_t", tag="x")
        nc.sync.dma_start(x_t[:], xt[:, g0 * RW : (g0 + SB) * RW])

        oh_t = ohp.tile([128, SB * BW], dtb, name="oh_t", tag="oh")
        ohb_t = ohp.tile([128, SB * 2 * BW], dtb, name="ohb_t", tag="ohb")

        oh3 = oh_t[:].rearrange("p (j c) -> p j c", c=BW)
        i3 = iota_t[:].unsqueeze(1).broadcast_to((128, SB, BW))
        t3 = tcols_t[:, g0 : g0 + SB].unsqueeze(2).broadcast_to((128, SB, BW))
        nc.vector.tensor_tensor(oh3, i3, t3, AOp.is_equal)

        ohb3 = ohb_t[:].rearrange("p (j c) -> p j c", c=2 * BW)
        rs3 = rs_t[:, g0 : g0 + SB].unsqueeze(2).broadcast_to((128, SB, BW))
        sw3 = sw_t[:, g0 : g0 + SB].unsqueeze(2).broadcast_to((128, SB, BW))
        nc.vector.tensor_tensor(ohb3[:, :, 0:BW], oh3, rs3, AOp.mult)
        nc.vector.tensor_tensor(ohb3[:, :, BW : 2 * BW], oh3, sw3, AOp.mult)

        xonly = x_t[:].rearrange("p (j c) -> p j c", c=RW)[:, :, 0:128]
        if s % 4 != 3:
            nc.scalar.activation(
                sq_scr3[s], xonly, AF.Square, accum_out=stats_t[:, s : s + 1]
            )
        else:
            # offload some square-accums to the vector engine
            nc.vector.scalar_tensor_tensor(
                sq_scr3[s], xonly, 1.0, xonly, AOp.mult, AOp.mult,
                accum_out=stats_t[:, s : s + 1],
            )

        for j in range(SB):
            g = g0 + j
            b = g // NBLK
            lb = g % NBLK
            w2 = 2 * BW
            nc.tensor.matmul(
                st_ps[b][:, 0:RW],
                ohb_t[:, j * w2 : (j + 1) * w2],
                x_t[:, j * RW : (j + 1) * RW],
                start=(lb == 0),
                stop=(lb == NBLK - 1),
            )


def _build_nc(loop_reps=None):
    import concourse.mybir as mybir
    import concourse.tile as tile
    from concourse import bacc

    dtb = mybir.dt.bfloat16
    dtf = mybir.dt.float32
    nc = bacc.Bacc(None, target_bir_lowering=False, debug=False)

    xt = nc.dram_tensor("xt", [128, TOT * 130], dtb, kind="ExternalInput")
    tcol = nc.dram_tensor("tcols", [128, TOT], dtb, kind="ExternalInput")
    rscol = nc.dram_tensor("rscols", [128, TOT], dtb, kind="ExternalInput")
    swcol = nc.dram_tensor("swcols", [128, TOT], dtb, kind="ExternalInput")
    iota = nc.dram_tensor("iota", [128, BW], dtb, kind="ExternalInput")
    o_st = nc.dram_tensor("o_st", [2 * BW, NBUCK * 130], dtf, kind="ExternalOutput")
    o_stats = nc.dram_tensor("o_stats", [128, NST], dtf, kind="ExternalOutput")

    with tile.TileContext(nc) as tc:
        with (
            tc.tile_pool(name="const", bufs=1) as constp,
            tc.tile_pool(name="xp", bufs=3) as xp,
            tc.tile_pool(name="ohp", bufs=3) as ohp,
            tc.tile_pool(name="scr", bufs=1) as scrp,
            tc.tile_pool(name="psum", bufs=1, space="PSUM") as pp,
            tc.tile_pool(name="outp", bufs=1) as outp,
        ):
            tcols_t = constp.tile([128, TOT], dtb, tag="tcols")
            nc.sync.dma_start(tcols_t[:], tcol[:])
            rs_t = constp.tile([128, TOT], dtb, tag="rs")
            nc.sync.dma_start(rs_t[:], rscol[:])
            sw_t = constp.tile([128, TOT], dtb, tag="sw")
            nc.sync.dma_start(sw_t[:], swcol[:])
            iota_t = constp.tile([128, BW], dtb, tag="iota")
            nc.sync.dma_start(iota_t[:], iota[:])
            stats_t = constp.tile([128, NST], dtf, tag="stats")

            st_ps = [
                pp.tile([2 * BW, 130], dtf, name=f"st{b}", tag=f"st{b}")
                for b in range(NBUCK)
            ]

            sq_scr = scrp.tile([128, SB * 128], dtb, tag="sq")
            sq_scr2 = scrp.tile([128, SB * 128], dtb, tag="sq2")
            sq_v = sq_scr[:].rearrange("p (j c) -> p j c", c=128)
            sq_v2 = sq_scr2[:].rearrange("p (j c) -> p j c", c=128)
            sq_scr3 = [sq_v if s % 4 != 3 else sq_v2 for s in range(NST)]

            loop_cm = (
                tc.For_i(0, loop_reps, 1)
                if loop_reps is not None
                else contextlib.nullcontext()
            )
            with loop_cm:
                _emit_body(nc, mybir, xt, tcols_t, rs_t, sw_t, iota_t,
                           stats_t, st_ps, sq_scr3, sq_scr, sq_scr2, xp, ohp)

            st_out = outp.tile([2 * BW, NBUCK * 130], dtf, tag="st_out")
            for b in range(NBUCK):
                nc.vector.tensor_copy(
                    st_out[:, b * 130 : (b + 1) * 130], st_ps[b][:]
                )
            nc.sync.dma_start(o_st[:], st_out[:])
            nc.sync.dma_start(o_stats[:], stats_t[:])

    nc.finalize()
    return nc


def _get_nc():
    global _CACHED_NC
    if _CACHED_NC is None:
        _CACHED_NC = _build_nc()
    return _CACHED_NC


def _prepare_inputs(x, t, w):
    """Bucket rows by class range, pad, prescale, transpose to device layout."""
    sw = np.sqrt(np.maximum(w, 1e-24), dtype=np.float32)
    rs = (1.0 / sw).astype(np.float32)

    gb = t // BW  # global bucket 0..31
    order = np.argsort(gb, kind="stable")
    counts = np.bincount(gb, minlength=NCORES * NBUCK)
    if counts.max() > CAP:
        raise RuntimeError(f"bucket overflow: {counts.max()} > {CAP}")

    GB = NCORES * NBUCK
    xs = x[order] * sw[order, None]  # f32 [N, D]
    ts = (t[order] % BW).astype(np.float32)
    sws = sw[order]
    rss = rs[order]
    ws = w[order]

    RW = 130
    Xp = np.zeros((GB, CAP, RW), dtype=BF16)
    Tp = np.zeros((GB, CAP), dtype=BF16)
    Sp = np.zeros((GB, CAP), dtype=BF16)
    Rp = np.zeros((GB, CAP), dtype=BF16)
    off = 0
    for g in range(GB):
        cnt = int(counts[g])
        seg = slice(off, off + cnt)
        Xp[g, :cnt, :D] = xs[seg].astype(BF16)
        Xp[g, :cnt, D] = sws[seg].astype(BF16)  # v * sw (v=1 for real rows)
        Xp[g, :cnt, D + 1] = (ws[seg] * sws[seg]).astype(BF16)  # w * sw
        Tp[g, :cnt] = ts[seg].astype(BF16)
        Sp[g, :cnt] = sws[seg].astype(BF16)
        Rp[g, :cnt] = rss[seg].astype(BF16)
        off += cnt

    iota_arr = np.tile(np.arange(BW, dtype=np.float32), (128, 1)).astype(BF16)

    in_maps = []
    for k in range(NCORES):
        sl = slice(NBUCK * k, NBUCK * (k + 1))
        xt_k = np.ascontiguousarray(
            Xp[sl].reshape(TOT, 128, RW).transpose(1, 0, 2).reshape(128, TOT * RW)
        )
        tc_k = np.ascontiguousarray(Tp[sl].reshape(TOT, 128).T)
        sw_k = np.ascontiguousarray(Sp[sl].reshape(TOT, 128).T)
        rs_k = np.ascontiguousarray(Rp[sl].reshape(TOT, 128).T)
        in_maps.append(
            {
                "xt": xt_k,
                "tcols": tc_k,
                "rscols": rs_k,
                "swcols": sw_k,
                "iota": iota_arr,
            }
        )
    return in_maps


def _combine(results):
    S = np.zeros((CSLOTS, D), dtype=np.float64)
    T = np.zeros((CSLOTS, D), dtype=np.float64)
    n = np.zeros(CSLOTS, dtype=np.float64)
    W = np.zeros(CSLOTS, dtype=np.float64)
    A = 0.0
    for k in range(NCORES):
        r = results[k]
        ost = np.asarray(r["o_st"], dtype=np.float64)
        A += float(np.asarray(r["o_stats"], dtype=np.float64).sum())
        for b in range(NBUCK):
            c0 = 128 * k + BW * b
            blk = ost[:, 130 * b : 130 * (b + 1)]
            S[c0 : c0 + BW] = blk[0:BW, 0:D]
            T[c0 : c0 + BW] = blk[BW : 2 * BW, 0:D]
            n[c0 : c0 + BW] = blk[0:BW, D]
            W[c0 : c0 + BW] = blk[0:BW, D + 1]

    n_int = np.round(n)
    means = S / np.maximum(n_int, 1.0)[:, None]
    Wsum = W.sum()
    total = A - 2.0 * float((means * T).sum()) + float(
        (W * (means * means).sum(axis=1)).sum()
    )
    return np.float32(total / Wsum)


def kernel(inputs, targets, weights, num_classes):
    from concourse.bass_utils import run_bass_kernel_spmd

    x = np.asarray(inputs, dtype=np.float32)
    t = np.asarray(targets).astype(np.int64)
    w = np.asarray(weights, dtype=np.float32)
    assert int(num_classes) == C, f"compiled for {C} classes, got {num_classes}"
    assert x.shape == (N, D) and t.shape == (N,) and w.shape == (N,)

    in_maps = _prepare_inputs(x, t, w)
    nc = _get_nc()
    res = run_bass_kernel_spmd(nc, in_maps, list(range(NCORES)))
    return _combine(res.results)


if __name__ == "__main__":
    rng = np.random.default_rng(0)
    x = rng.standard_normal((N, D)).astype(np.float32)
    t = rng.integers(0, C, N).astype(np.int64)
    w = rng.random(N).astype(np.float32)
    out = kernel(x, t, w, C)
    print("kernel output:", out)


# revision 9
# speedup vs baseline: 1.1591x; 1.0476x over previous
"""Trainium2 Bass kernel for nn_RegLoss (segment-reduce weighted regression loss).

Math: with per-class means m_c = S_c / max(n_c, 1), S_c = sum_{i: t_i=c} x_i,
    loss = sum_i w_i * ||x_i - m_{t_i}||^2 / sum_i w_i
         = (A - 2*sum_c m_c.T_c + sum_c W_c*||m_c||^2) / sum_i w_i
with A = sum_i w_i ||x_i||^2, T_c = sum_{i in c} w_i x_i, W_c = sum_{i in c} w_i.
Everything reduces to per-class segment sums + one global weighted square sum.

Sharding: rows are bucketed by class range (32 classes per bucket, 4 buckets
per core -> core k owns classes [128k, 128k+128)), padded to a fixed per-bucket
capacity. Per 128-row block the device builds a [128,32] one-hot from the local
class index and accumulates, via TensorE matmuls into PSUM:
  out12[d, c|c+32] += xsw_block.T @ [oh*rs | oh*sw]   -> S_c, T_c
  out3 [0:2, c]    += [v*sw, w*sw].T @ (oh*rs)        -> n_c, W_c
where xsw = sqrt(w)*x is the (host-prescaled) data, rs = 1/sqrt(w).  The
prescaling makes A = sum(xsw^2) an unweighted square-sum, done on ScalarE
(Square activation with accum_out).  Host combines the per-core partials.
"""

import contextlib
import sys

for _p in ("/opt/trn_rl_repo",):
    if _p not in sys.path:
        sys.path.insert(0, _p)

import numpy as np
import ml_dtypes

BF16 = ml_dtypes.bfloat16

# Problem constants (hardcoded per contract)
N = 500000
D = 128
C = 1000
NCORES = 8
BW = 16                 # classes per bucket
NBUCK = 8               # buckets per core
CSLOTS = NCORES * NBUCK * BW  # 1024 padded class slots
CAP = 8320              # padded rows per bucket (max observed 8172)
NBLK = CAP // 128       # blocks per bucket = 65
TOT = NBUCK * NBLK      # blocks per core = 520
SB = 40                 # blocks per supertile
NST = TOT // SB         # supertiles per core = 13

_CACHED_NC = None


def _emit_body(nc, mybir, xt, tcols_t, rs_t, sw_t, iota_t, stats_t, stats2_t,
               st_ps, sq_scr3, sqj_scr3, xp, ohp):
    AOp = mybir.AluOpType
    AF = mybir.ActivationFunctionType
    dtb = mybir.dt.bfloat16
    RW = 130  # per-block rhs width: 128 x cols + vsw + wsw
    for s in range(NST):
        g0 = s * SB
        x_t = xp.tile([128, SB * RW], dtb, name="x_t", tag="x")
        nc.sync.dma_start(x_t[:], xt[:, g0 * RW : (g0 + SB) * RW])

        oh_t = ohp.tile([128, SB * BW], dtb, name="oh_t", tag="oh")
        ohb_t = ohp.tile([128, SB * 2 * BW], dtb, name="ohb_t", tag="ohb")

        oh3 = oh_t[:].rearrange("p (j c) -> p j c", c=BW)
        i3 = iota_t[:].unsqueeze(1).broadcast_to((128, SB, BW))
        t3 = tcols_t[:, g0 : g0 + SB].unsqueeze(2).broadcast_to((128, SB, BW))
        nc.vector.tensor_tensor(oh3, i3, t3, AOp.is_equal)

        ohb3 = ohb_t[:].rearrange("p (j c) -> p j c", c=2 * BW)
        rs3 = rs_t[:, g0 : g0 + SB].unsqueeze(2).broadcast_to((128, SB, BW))
        sw3 = sw_t[:, g0 : g0 + SB].unsqueeze(2).broadcast_to((128, SB, BW))
        nc.vector.tensor_tensor(ohb3[:, :, 0:BW], oh3, rs3, AOp.mult)
        nc.vector.tensor_tensor(ohb3[:, :, BW : 2 * BW], oh3, sw3, AOp.mult)

        # square the full contiguous stream (incl. the 2 aux cols per block);
        # the aux contribution sum(vsw^2 + wsw^2) is accumulated separately
        # below and subtracted on the host.
        if s % 4 != 3:
            nc.scalar.activation(
                sq_scr3[s], x_t[:], AF.Square, accum_out=stats_t[:, s : s + 1]
            )
        else:
            # offload some square-accums to the vector engine
            nc.vector.scalar_tensor_tensor(
                sq_scr3[s], x_t[:], 1.0, x_t[:], AOp.mult, AOp.mult,
                accum_out=stats_t[:, s : s + 1],
            )
        aux = x_t[:].rearrange("p (j c) -> p j c", c=RW)[:, :, 128:RW]
        nc.scalar.activation(
            sqj_scr3, aux, AF.Square, accum_out=stats2_t[:, s : s + 1]
        )

        for j in range(SB):
            g = g0 + j
            b = g // NBLK
            lb = g % NBLK
            w2 = 2 * BW
            nc.tensor.matmul(
                st_ps[b][:, 0:RW],
                ohb_t[:, j * w2 : (j + 1) * w2],
                x_t[:, j * RW : (j + 1) * RW],
                start=(lb == 0),
                stop=(lb == NBLK - 1),
            )


def _build_nc(loop_reps=None):
    import concourse.mybir as mybir
    import concourse.tile as tile
    from concourse import bacc

    dtb = mybir.dt.bfloat16
    dtf = mybir.dt.float32
    nc = bacc.Bacc(None, target_bir_lowering=False, debug=False)

    xt = nc.dram_tensor("xt", [128, TOT * 130], dtb, kind="ExternalInput")
    tcol = nc.dram_tensor("tcols", [128, TOT], dtb, kind="ExternalInput")
    rscol = nc.dram_tensor("rscols", [128, TOT], dtb, kind="ExternalInput")
    swcol = nc.dram_tensor("swcols", [128, TOT], dtb, kind="ExternalInput")
    iota = nc.dram_tensor("iota", [128, BW], dtb, kind="ExternalInput")
    o_st = nc.dram_tensor("o_st", [2 * BW, NBUCK * 130], dtf, kind="ExternalOutput")
    o_stats = nc.dram_tensor("o_stats", [128, NST], dtf, kind="ExternalOutput")
    o_stats2 = nc.dram_tensor("o_stats2", [128, NST], dtf, kind="ExternalOutput")

    with tile.TileContext(nc) as tc:
        with (
            tc.tile_pool(name="const", bufs=1) as constp,
            tc.tile_pool(name="xp", bufs=4) as xp,
            tc.tile_pool(name="ohp", bufs=4) as ohp,
            tc.tile_pool(name="scr", bufs=1) as scrp,
            tc.tile_pool(name="psum", bufs=1, space="PSUM") as pp,
            tc.tile_pool(name="outp", bufs=1) as outp,
        ):
            tcols_t = constp.tile([128, TOT], dtb, tag="tcols")
            nc.sync.dma_start(tcols_t[:], tcol[:])
            rs_t = constp.tile([128, TOT], dtb, tag="rs")
            nc.sync.dma_start(rs_t[:], rscol[:])
            sw_t = constp.tile([128, TOT], dtb, tag="sw")
            nc.sync.dma_start(sw_t[:], swcol[:])
            iota_t = constp.tile([128, BW], dtb, tag="iota")
            nc.sync.dma_start(iota_t[:], iota[:])
            stats_t = constp.tile([128, NST], dtf, tag="stats")
            stats2_t = constp.tile([128, NST], dtf, tag="stats2")

            st_ps = [
                pp.tile([2 * BW, 130], dtf, name=f"st{b}", tag=f"st{b}")
                for b in range(NBUCK)
            ]

            sq_scr = scrp.tile([128, SB * 130], dtb, tag="sq")
            sq_scr2 = scrp.tile([128, SB * 130], dtb, tag="sq2")
            sq_scr3 = [sq_scr[:] if s % 4 != 3 else sq_scr2[:] for s in range(NST)]
            sqj_scr = scrp.tile([128, SB * 2], dtb, tag="sqj")
            sqj_scr3 = sqj_scr[:].rearrange("p (j c) -> p j c", c=2)

            loop_cm = (
                tc.For_i(0, loop_reps, 1, hint_engines=(mybir.EngineType.PE,))
                if loop_reps is not None
                else contextlib.nullcontext()
            )
            with loop_cm:
                _emit_body(nc, mybir, xt, tcols_t, rs_t, sw_t, iota_t,
                           stats_t, stats2_t, st_ps, sq_scr3, sqj_scr3, xp, ohp)

            st_out = outp.tile([2 * BW, NBUCK * 130], dtf, tag="st_out")
            for b in range(NBUCK):
                nc.vector.tensor_copy(
                    st_out[:, b * 130 : (b + 1) * 130], st_ps[b][:]
                )
            nc.sync.dma_start(o_st[:], st_out[:])
            nc.sync.dma_start(o_stats[:], stats_t[:])
            nc.sync.dma_start(o_stats2[:], stats2_t[:])

    nc.finalize()
    return nc


def _get_nc():
    global _CACHED_NC
    if _CACHED_NC is None:
        _CACHED_NC = _build_nc()
    return _CACHED_NC


def _prepare_inputs(x, t, w):
    """Bucket rows by class range, pad, prescale, transpose to device layout."""
    sw = np.sqrt(np.maximum(w, 1e-24), dtype=np.float32)
    rs = (1.0 / sw).astype(np.float32)

    gb = t // BW  # global bucket 0..31
    order = np.argsort(gb, kind="stable")
    counts = np.bincount(gb, minlength=NCORES * NBUCK)
    if counts.max() > CAP:
        raise RuntimeError(f"bucket overflow: {counts.max()} > {CAP}")

    GB = NCORES * NBUCK
    xs = x[order] * sw[order, None]  # f32 [N, D]
    ts = (t[order] % BW).astype(np.float32)
    sws = sw[order]
    rss = rs[order]
    ws = w[order]

    RW = 130
    Xp = np.zeros((GB, CAP, RW), dtype=BF16)
    Tp = np.zeros((GB, CAP), dtype=BF16)
    Sp = np.zeros((GB, CAP), dtype=BF16)
    Rp = np.zeros((GB, CAP), dtype=BF16)
    off = 0
    for g in range(GB):
        cnt = int(counts[g])
        seg = slice(off, off + cnt)
        Xp[g, :cnt, :D] = xs[seg].astype(BF16)
        Xp[g, :cnt, D] = sws[seg].astype(BF16)  # v * sw (v=1 for real rows)
        Xp[g, :cnt, D + 1] = (ws[seg] * sws[seg]).astype(BF16)  # w * sw
        Tp[g, :cnt] = ts[seg].astype(BF16)
        Sp[g, :cnt] = sws[seg].astype(BF16)
        Rp[g, :cnt] = rss[seg].astype(BF16)
        off += cnt

    iota_arr = np.tile(np.arange(BW, dtype=np.float32), (128, 1)).astype(BF16)

    in_maps = []
    for k in range(NCORES):
        sl = slice(NBUCK * k, NBUCK * (k + 1))
        xt_k = np.ascontiguousarray(
            Xp[sl].reshape(TOT, 128, RW).transpose(1, 0, 2).reshape(128, TOT * RW)
        )
        tc_k = np.ascontiguousarray(Tp[sl].reshape(TOT, 128).T)
        sw_k = np.ascontiguousarray(Sp[sl].reshape(TOT, 128).T)
        rs_k = np.ascontiguousarray(Rp[sl].reshape(TOT, 128).T)
        in_maps.append(
            {
                "xt": xt_k,
                "tcols": tc_k,
                "rscols": rs_k,
                "swcols": sw_k,
                "iota": iota_arr,
            }
        )
    return in_maps


def _combine(results):
    S = np.zeros((CSLOTS, D), dtype=np.float64)
    T = np.zeros((CSLOTS, D), dtype=np.float64)
    n = np.zeros(CSLOTS, dtype=np.float64)
    W = np.zeros(CSLOTS, dtype=np.float64)
    A = 0.0
    for k in range(NCORES):
        r = results[k]
        ost = np.asarray(r["o_st"], dtype=np.float64)
        A += float(np.asarray(r["o_stats"], dtype=np.float64).sum())
        A -= float(np.asarray(r["o_stats2"], dtype=np.float64).sum())
        for b in range(NBUCK):
            c0 = 128 * k + BW * b
            blk = ost[:, 130 * b : 130 * (b + 1)]
            S[c0 : c0 + BW] = blk[0:BW, 0:D]
            T[c0 : c0 + BW] = blk[BW : 2 * BW, 0:D]
            n[c0 : c0 + BW] = blk[0:BW, D]
            W[c0 : c0 + BW] = blk[0:BW, D + 1]

    n_int = np.round(n)
    means = S / np.maximum(n_int, 1.0)[:, None]
    Wsum = W.sum()
    total = A - 2.0 * float((means * T).sum()) + float(
        (W * (means * means).sum(axis=1)).sum()
    )
    return np.float32(total / Wsum)


def kernel(inputs, targets, weights, num_classes):
    from concourse.bass_utils import run_bass_kernel_spmd

    x = np.asarray(inputs, dtype=np.float32)
    t = np.asarray(targets).astype(np.int64)
    w = np.asarray(weights, dtype=np.float32)
    assert int(num_classes) == C, f"compiled for {C} classes, got {num_classes}"
    assert x.shape == (N, D) and t.shape == (N,) and w.shape == (N,)

    in_maps = _prepare_inputs(x, t, w)
    nc = _get_nc()
    res = run_bass_kernel_spmd(nc, in_maps, list(range(NCORES)))
    return _combine(res.results)


if __name__ == "__main__":
    rng = np.random.default_rng(0)
    x = rng.standard_normal((N, D)).astype(np.float32)
    t = rng.integers(0, C, N).astype(np.int64)
    w = rng.random(N).astype(np.float32)
    out = kernel(x, t, w, C)
    print("kernel output:", out)


# revision 10
# speedup vs baseline: 1.3687x; 1.1808x over previous
"""Trainium2 Bass kernel for nn_RegLoss (segment-reduce weighted regression loss).

Math: with per-class means m_c = S_c / max(n_c, 1), S_c = sum_{i: t_i=c} x_i,
    loss = sum_i w_i * ||x_i - m_{t_i}||^2 / sum_i w_i
         = (A - 2*sum_c m_c.T_c + sum_c W_c*||m_c||^2) / sum_i w_i
with A = sum_i w_i ||x_i||^2, T_c = sum_{i in c} w_i x_i, W_c = sum_{i in c} w_i.
Everything reduces to per-class segment sums + one global weighted square sum.

Sharding: rows are bucketed by class range (32 classes per bucket, 4 buckets
per core -> core k owns classes [128k, 128k+128)), padded to a fixed per-bucket
capacity. Per 128-row block the device builds a [128,32] one-hot from the local
class index and accumulates, via TensorE matmuls into PSUM:
  out12[d, c|c+32] += xsw_block.T @ [oh*rs | oh*sw]   -> S_c, T_c
  out3 [0:2, c]    += [v*sw, w*sw].T @ (oh*rs)        -> n_c, W_c
where xsw = sqrt(w)*x is the (host-prescaled) data, rs = 1/sqrt(w).  The
prescaling makes A = sum(xsw^2) an unweighted square-sum, done on ScalarE
(Square activation with accum_out).  Host combines the per-core partials.
"""

import contextlib
import sys

for _p in ("/opt/trn_rl_repo",):
    if _p not in sys.path:
        sys.path.insert(0, _p)

import numpy as np
import ml_dtypes

BF16 = ml_dtypes.bfloat16

# Problem constants (hardcoded per contract)
N = 500000
D = 128
C = 1000
NCORES = 8
BW = 16                 # classes per bucket
NBUCK = 8               # buckets per core
CSLOTS = NCORES * NBUCK * BW  # 1024 padded class slots
CAP = 8320              # padded rows per bucket (max observed 8172)
NBLK = CAP // 128       # blocks per bucket = 65
TOT = NBUCK * NBLK      # blocks per core = 520
SB = 40                 # blocks per supertile
NST = TOT // SB         # supertiles per core = 13

_CACHED_NC = None


def _emit_body(nc, mybir, xt, tcols_t, rssw_t, iota_t, stats_t,
               st_ps, sq_scr3, xp, ohp):
    AOp = mybir.AluOpType
    AF = mybir.ActivationFunctionType
    dtb = mybir.dt.bfloat16
    RW = 130  # per-block rhs width: 128 x cols + vsw + wsw
    for s in range(NST):
        g0 = s * SB
        x_t = xp.tile([128, SB * RW], dtb, name="x_t", tag="x")
        nc.sync.dma_start(x_t[:], xt[:, g0 * RW : (g0 + SB) * RW])

        oh_t = ohp.tile([128, SB * BW], dtb, name="oh_t", tag="oh")
        ohb_t = ohp.tile([128, SB * 2 * BW], dtb, name="ohb_t", tag="ohb")

        oh3 = oh_t[:].rearrange("p (j c) -> p j c", c=BW)
        i3 = iota_t[:].unsqueeze(1).broadcast_to((128, SB, BW))
        t3 = tcols_t[:, g0 : g0 + SB].unsqueeze(2).broadcast_to((128, SB, BW))
        nc.vector.tensor_tensor(oh3, i3, t3, AOp.is_equal)

        ohb4 = ohb_t[:].rearrange("p (j h c) -> p j h c", h=2, c=BW)
        oh4 = oh3.unsqueeze(2).broadcast_to((128, SB, 2, BW))
        rssw4 = (
            rssw_t[:, 2 * g0 : 2 * (g0 + SB)]
            .rearrange("p (j h) -> p j h", h=2)
            .unsqueeze(3)
            .broadcast_to((128, SB, 2, BW))
        )
        nc.vector.tensor_tensor(ohb4, oh4, rssw4, AOp.mult)

        # square the full contiguous stream (incl. the 2 aux cols per block);
        # the aux contribution sum(vsw^2 + wsw^2) is accumulated separately
        # below and subtracted on the host.
        if s % 4 != 3:
            nc.scalar.activation(
                sq_scr3[s], x_t[:], AF.Square, accum_out=stats_t[:, s : s + 1]
            )
        else:
            # offload some square-accums to the vector engine
            nc.vector.scalar_tensor_tensor(
                sq_scr3[s], x_t[:], 1.0, x_t[:], AOp.mult, AOp.mult,
                accum_out=stats_t[:, s : s + 1],
            )

        for j in range(SB):
            g = g0 + j
            b = g // NBLK
            lb = g % NBLK
            w2 = 2 * BW
            nc.tensor.matmul(
                st_ps[b][:, 0:RW],
                ohb_t[:, j * w2 : (j + 1) * w2],
                x_t[:, j * RW : (j + 1) * RW],
                start=(lb == 0),
                stop=(lb == NBLK - 1),
            )


def _build_nc(loop_reps=None):
    import concourse.mybir as mybir
    import concourse.tile as tile
    from concourse import bacc

    dtb = mybir.dt.bfloat16
    dtf = mybir.dt.float32
    nc = bacc.Bacc(None, target_bir_lowering=False, debug=False)

    xt = nc.dram_tensor("xt", [128, TOT * 130], dtb, kind="ExternalInput")
    tcol = nc.dram_tensor("tcols", [128, TOT], dtb, kind="ExternalInput")
    rssw = nc.dram_tensor("rsswcols", [128, TOT * 2], dtb, kind="ExternalInput")
    iota = nc.dram_tensor("iota", [128, BW], dtb, kind="ExternalInput")
    o_st = nc.dram_tensor("o_st", [2 * BW, NBUCK * 130], dtf, kind="ExternalOutput")
    o_stats = nc.dram_tensor("o_stats", [128, NST], dtf, kind="ExternalOutput")

    with tile.TileContext(nc) as tc:
        with (
            tc.tile_pool(name="const", bufs=1) as constp,
            tc.tile_pool(name="xp", bufs=4) as xp,
            tc.tile_pool(name="ohp", bufs=4) as ohp,
            tc.tile_pool(name="scr", bufs=1) as scrp,
            tc.tile_pool(name="psum", bufs=1, space="PSUM") as pp,
            tc.tile_pool(name="outp", bufs=1) as outp,
        ):
            tcols_t = constp.tile([128, TOT], dtb, tag="tcols")
            nc.sync.dma_start(tcols_t[:], tcol[:])
            rssw_t = constp.tile([128, TOT * 2], dtb, tag="rssw")
            nc.sync.dma_start(rssw_t[:], rssw[:])
            iota_t = constp.tile([128, BW], dtb, tag="iota")
            nc.sync.dma_start(iota_t[:], iota[:])
            stats_t = constp.tile([128, NST], dtf, tag="stats")

            st_ps = [
                pp.tile([2 * BW, 130], dtf, name=f"st{b}", tag=f"st{b}")
                for b in range(NBUCK)
            ]

            sq_scr = scrp.tile([128, SB * 130], dtb, tag="sq")
            sq_scr2 = scrp.tile([128, SB * 130], dtb, tag="sq2")
            sq_scr3 = [sq_scr[:] if s % 4 != 3 else sq_scr2[:] for s in range(NST)]

            loop_cm = (
                tc.For_i(0, loop_reps, 1, hint_engines=(mybir.EngineType.PE,))
                if loop_reps is not None
                else contextlib.nullcontext()
            )
            with loop_cm:
                _emit_body(nc, mybir, xt, tcols_t, rssw_t, iota_t,
                           stats_t, st_ps, sq_scr3, xp, ohp)

            st_out = outp.tile([2 * BW, NBUCK * 130], dtf, tag="st_out")
            for b in range(NBUCK):
                nc.vector.tensor_copy(
                    st_out[:, b * 130 : (b + 1) * 130], st_ps[b][:]
                )
            nc.sync.dma_start(o_st[:], st_out[:])
            nc.sync.dma_start(o_stats[:], stats_t[:])

    nc.finalize()
    return nc


def _get_nc():
    global _CACHED_NC
    if _CACHED_NC is None:
        _CACHED_NC = _build_nc()
    return _CACHED_NC


def _prepare_inputs(x, t, w):
    """Bucket rows by class range, pad, prescale, transpose to device layout."""
    sw = np.sqrt(np.maximum(w, 1e-24), dtype=np.float32)
    rs = (1.0 / sw).astype(np.float32)

    gb = t // BW  # global bucket 0..31
    order = np.argsort(gb, kind="stable")
    counts = np.bincount(gb, minlength=NCORES * NBUCK)
    if counts.max() > CAP:
        raise RuntimeError(f"bucket overflow: {counts.max()} > {CAP}")

    GB = NCORES * NBUCK
    xs = x[order] * sw[order, None]  # f32 [N, D]
    ts = (t[order] % BW).astype(np.float32)
    sws = sw[order]
    rss = rs[order]
    ws = w[order]

    RW = 130
    Xp = np.zeros((GB, CAP, RW), dtype=BF16)
    Tp = np.zeros((GB, CAP), dtype=BF16)
    RSp = np.zeros((GB, CAP, 2), dtype=BF16)
    off = 0
    for g in range(GB):
        cnt = int(counts[g])
        seg = slice(off, off + cnt)
        Xp[g, :cnt, :D] = xs[seg].astype(BF16)
        Xp[g, :cnt, D] = sws[seg].astype(BF16)  # v * sw (v=1 for real rows)
        Xp[g, :cnt, D + 1] = (ws[seg] * sws[seg]).astype(BF16)  # w * sw
        Tp[g, :cnt] = ts[seg].astype(BF16)
        RSp[g, :cnt, 0] = rss[seg].astype(BF16)
        RSp[g, :cnt, 1] = sws[seg].astype(BF16)
        off += cnt

    iota_arr = np.tile(np.arange(BW, dtype=np.float32), (128, 1)).astype(BF16)
    aux = Xp[:, :, D : D + 2].astype(np.float64)
    wcorr = float((aux * aux).sum())

    in_maps = []
    for k in range(NCORES):
        sl = slice(NBUCK * k, NBUCK * (k + 1))
        xt_k = np.ascontiguousarray(
            Xp[sl].reshape(TOT, 128, RW).transpose(1, 0, 2).reshape(128, TOT * RW)
        )
        tc_k = np.ascontiguousarray(Tp[sl].reshape(TOT, 128).T)
        rssw_k = np.ascontiguousarray(
            RSp[sl].reshape(TOT, 128, 2).transpose(1, 0, 2).reshape(128, TOT * 2)
        )
        in_maps.append(
            {
                "xt": xt_k,
                "tcols": tc_k,
                "rsswcols": rssw_k,
                "iota": iota_arr,
            }
        )
    return in_maps, wcorr


def _combine(results, wcorr):
    S = np.zeros((CSLOTS, D), dtype=np.float64)
    T = np.zeros((CSLOTS, D), dtype=np.float64)
    n = np.zeros(CSLOTS, dtype=np.float64)
    W = np.zeros(CSLOTS, dtype=np.float64)
    A = 0.0
    for k in range(NCORES):
        r = results[k]
        ost = np.asarray(r["o_st"], dtype=np.float64)
        A += float(np.asarray(r["o_stats"], dtype=np.float64).sum())
        for b in range(NBUCK):
            c0 = 128 * k + BW * b
            blk = ost[:, 130 * b : 130 * (b + 1)]
            S[c0 : c0 + BW] = blk[0:BW, 0:D]
            T[c0 : c0 + BW] = blk[BW : 2 * BW, 0:D]
            n[c0 : c0 + BW] = blk[0:BW, D]
            W[c0 : c0 + BW] = blk[0:BW, D + 1]

    A -= wcorr
    n_int = np.round(n)
    means = S / np.maximum(n_int, 1.0)[:, None]
    Wsum = W.sum()
    total = A - 2.0 * float((means * T).sum()) + float(
        (W * (means * means).sum(axis=1)).sum()
    )
    return np.float32(total / Wsum)


def kernel(inputs, targets, weights, num_classes):
    from concourse.bass_utils import run_bass_kernel_spmd

    x = np.asarray(inputs, dtype=np.float32)
    t = np.asarray(targets).astype(np.int64)
    w = np.asarray(weights, dtype=np.float32)
    assert int(num_classes) == C, f"compiled for {C} classes, got {num_classes}"
    assert x.shape == (N, D) and t.shape == (N,) and w.shape == (N,)

    in_maps, wcorr = _prepare_inputs(x, t, w)
    nc = _get_nc()
    res = run_bass_kernel_spmd(nc, in_maps, list(range(NCORES)))
    return _combine(res.results, wcorr)


if __name__ == "__main__":
    rng = np.random.default_rng(0)
    x = rng.standard_normal((N, D)).astype(np.float32)
    t = rng.integers(0, C, N).astype(np.int64)
    w = rng.random(N).astype(np.float32)
    out = kernel(x, t, w, C)
    print("kernel output:", out)


# revision 11
# speedup vs baseline: 1.3694x; 1.0005x over previous
"""Trainium2 Bass kernel for nn_RegLoss (segment-reduce weighted regression loss).

Math: with per-class means m_c = S_c / max(n_c, 1), S_c = sum_{i: t_i=c} x_i,
    loss = sum_i w_i * ||x_i - m_{t_i}||^2 / sum_i w_i
         = (A - 2*sum_c m_c.T_c + sum_c W_c*||m_c||^2) / sum_i w_i
with A = sum_i w_i ||x_i||^2, T_c = sum_{i in c} w_i x_i, W_c = sum_{i in c} w_i.
Everything reduces to per-class segment sums + one global weighted square sum.

Sharding: rows are bucketed by class range (32 classes per bucket, 4 buckets
per core -> core k owns classes [128k, 128k+128)), padded to a fixed per-bucket
capacity. Per 128-row block the device builds a [128,32] one-hot from the local
class index and accumulates, via TensorE matmuls into PSUM:
  out12[d, c|c+32] += xsw_block.T @ [oh*rs | oh*sw]   -> S_c, T_c
  out3 [0:2, c]    += [v*sw, w*sw].T @ (oh*rs)        -> n_c, W_c
where xsw = sqrt(w)*x is the (host-prescaled) data, rs = 1/sqrt(w).  The
prescaling makes A = sum(xsw^2) an unweighted square-sum, done on ScalarE
(Square activation with accum_out).  Host combines the per-core partials.
"""

import contextlib
import sys

for _p in ("/opt/trn_rl_repo",):
    if _p not in sys.path:
        sys.path.insert(0, _p)

import numpy as np
import ml_dtypes

BF16 = ml_dtypes.bfloat16

# Problem constants (hardcoded per contract)
N = 500000
D = 128
C = 1000
NCORES = 8
BW = 16                 # classes per bucket
NBUCK = 8               # buckets per core
CSLOTS = NCORES * NBUCK * BW  # 1024 padded class slots
CAP = 8320              # padded rows per bucket (max observed 8172)
NBLK = CAP // 128       # blocks per bucket = 65
TOT = NBUCK * NBLK      # blocks per core = 520
SB = 40                 # blocks per supertile
NST = TOT // SB         # supertiles per core = 13

_CACHED_NC = None


def _emit_body(nc, mybir, xt, tcols_t, rssw_t, iota_t, stats_t,
               st_ps, sq_scr3, xp, ohp):
    AOp = mybir.AluOpType
    AF = mybir.ActivationFunctionType
    dtb = mybir.dt.bfloat16
    RW = 130  # per-block rhs width: 128 x cols + vsw + wsw
    for s in range(NST):
        g0 = s * SB
        x_t = xp.tile([128, SB * RW], dtb, name="x_t", tag="x")
        nc.sync.dma_start(x_t[:], xt[:, g0 * RW : (g0 + SB) * RW])

        oh_t = ohp.tile([128, SB * BW], dtb, name="oh_t", tag="oh")
        ohb_t = ohp.tile([128, SB * 2 * BW], dtb, name="ohb_t", tag="ohb")

        oh3 = oh_t[:].rearrange("p (j c) -> p j c", c=BW)
        i3 = iota_t[:].unsqueeze(1).broadcast_to((128, SB, BW))
        t3 = tcols_t[:, g0 : g0 + SB].unsqueeze(2).broadcast_to((128, SB, BW))
        nc.vector.tensor_tensor(oh3, i3, t3, AOp.is_equal)

        ohb4 = ohb_t[:].rearrange("p (j h c) -> p j h c", h=2, c=BW)
        oh4 = oh3.unsqueeze(2).broadcast_to((128, SB, 2, BW))
        rssw4 = (
            rssw_t[:, 2 * g0 : 2 * (g0 + SB)]
            .rearrange("p (j h) -> p j h", h=2)
            .unsqueeze(3)
            .broadcast_to((128, SB, 2, BW))
        )
        nc.vector.tensor_tensor(ohb4, oh4, rssw4, AOp.mult)

        # square the full contiguous stream (incl. the 2 aux cols per block);
        # the aux contribution sum(vsw^2 + wsw^2) is accumulated separately
        # below and subtracted on the host.
        if s % 4 != 3:
            nc.scalar.activation(
                sq_scr3[s], x_t[:], AF.Square, accum_out=stats_t[:, s : s + 1]
            )
        else:
            # offload some square-accums to the vector engine
            nc.vector.scalar_tensor_tensor(
                sq_scr3[s], x_t[:], 1.0, x_t[:], AOp.mult, AOp.mult,
                accum_out=stats_t[:, s : s + 1],
            )

        for j in range(SB):
            g = g0 + j
            b = g // NBLK
            lb = g % NBLK
            w2 = 2 * BW
            nc.tensor.matmul(
                st_ps[b][:, 0:RW],
                ohb_t[:, j * w2 : (j + 1) * w2],
                x_t[:, j * RW : (j + 1) * RW],
                start=(lb == 0),
                stop=(lb == NBLK - 1),
            )


def _build_nc(loop_reps=None):
    import concourse.mybir as mybir
    import concourse.tile as tile
    from concourse import bacc

    dtb = mybir.dt.bfloat16
    dtf = mybir.dt.float32
    nc = bacc.Bacc(None, target_bir_lowering=False, debug=False)

    xt = nc.dram_tensor("xt", [128, TOT * 130], dtb, kind="ExternalInput")
    tcol = nc.dram_tensor("tcols", [128, TOT], dtb, kind="ExternalInput")
    rssw = nc.dram_tensor("rsswcols", [128, TOT * 2], dtb, kind="ExternalInput")
    iota = nc.dram_tensor("iota", [128, BW], dtb, kind="ExternalInput")
    o_st = nc.dram_tensor("o_st", [2 * BW, NBUCK * 130], dtf, kind="ExternalOutput")
    o_stats = nc.dram_tensor("o_stats", [128, NST], dtf, kind="ExternalOutput")

    with tile.TileContext(nc) as tc:
        with (
            tc.tile_pool(name="const", bufs=1) as constp,
            tc.tile_pool(name="xp", bufs=6) as xp,
            tc.tile_pool(name="ohp", bufs=6) as ohp,
            tc.tile_pool(name="scr", bufs=1) as scrp,
            tc.tile_pool(name="psum", bufs=1, space="PSUM") as pp,
            tc.tile_pool(name="outp", bufs=1) as outp,
        ):
            tcols_t = constp.tile([128, TOT], dtb, tag="tcols")
            nc.sync.dma_start(tcols_t[:], tcol[:])
            rssw_t = constp.tile([128, TOT * 2], dtb, tag="rssw")
            nc.sync.dma_start(rssw_t[:], rssw[:])
            iota_t = constp.tile([128, BW], dtb, tag="iota")
            nc.sync.dma_start(iota_t[:], iota[:])
            stats_t = constp.tile([128, NST], dtf, tag="stats")

            st_ps = [
                pp.tile([2 * BW, 130], dtf, name=f"st{b}", tag=f"st{b}")
                for b in range(NBUCK)
            ]

            sq_scr = scrp.tile([128, SB * 130], dtb, tag="sq")
            sq_scr2 = scrp.tile([128, SB * 130], dtb, tag="sq2")
            sq_scr3 = [sq_scr[:] if s % 4 != 3 else sq_scr2[:] for s in range(NST)]

            loop_cm = (
                tc.For_i(0, loop_reps, 1, hint_engines=(mybir.EngineType.PE,))
                if loop_reps is not None
                else contextlib.nullcontext()
            )
            with loop_cm:
                _emit_body(nc, mybir, xt, tcols_t, rssw_t, iota_t,
                           stats_t, st_ps, sq_scr3, xp, ohp)

            st_out = outp.tile([2 * BW, NBUCK * 130], dtf, tag="st_out")
            for b in range(NBUCK):
                nc.vector.tensor_copy(
                    st_out[:, b * 130 : (b + 1) * 130], st_ps[b][:]
                )
            nc.sync.dma_start(o_st[:], st_out[:])
            nc.sync.dma_start(o_stats[:], stats_t[:])

    nc.finalize()
    return nc


def _get_nc():
    global _CACHED_NC
    if _CACHED_NC is None:
        _CACHED_NC = _build_nc()
    return _CACHED_NC


def _prepare_inputs(x, t, w):
    """Bucket rows by class range, pad, prescale, transpose to device layout."""
    sw = np.sqrt(np.maximum(w, 1e-24), dtype=np.float32)
    rs = (1.0 / sw).astype(np.float32)

    gb = t // BW  # global bucket 0..31
    order = np.argsort(gb, kind="stable")
    counts = np.bincount(gb, minlength=NCORES * NBUCK)
    if counts.max() > CAP:
        raise RuntimeError(f"bucket overflow: {counts.max()} > {CAP}")

    GB = NCORES * NBUCK
    xs = x[order] * sw[order, None]  # f32 [N, D]
    ts = (t[order] % BW).astype(np.float32)
    sws = sw[order]
    rss = rs[order]
    ws = w[order]

    RW = 130
    Xp = np.zeros((GB, CAP, RW), dtype=BF16)
    Tp = np.zeros((GB, CAP), dtype=BF16)
    RSp = np.zeros((GB, CAP, 2), dtype=BF16)
    off = 0
    for g in range(GB):
        cnt = int(counts[g])
        seg = slice(off, off + cnt)
        Xp[g, :cnt, :D] = xs[seg].astype(BF16)
        Xp[g, :cnt, D] = sws[seg].astype(BF16)  # v * sw (v=1 for real rows)
        Xp[g, :cnt, D + 1] = (ws[seg] * sws[seg]).astype(BF16)  # w * sw
        Tp[g, :cnt] = ts[seg].astype(BF16)
        RSp[g, :cnt, 0] = rss[seg].astype(BF16)
        RSp[g, :cnt, 1] = sws[seg].astype(BF16)
        off += cnt

    iota_arr = np.tile(np.arange(BW, dtype=np.float32), (128, 1)).astype(BF16)
    aux = Xp[:, :, D : D + 2].astype(np.float64)
    wcorr = float((aux * aux).sum())

    in_maps = []
    for k in range(NCORES):
        sl = slice(NBUCK * k, NBUCK * (k + 1))
        xt_k = np.ascontiguousarray(
            Xp[sl].reshape(TOT, 128, RW).transpose(1, 0, 2).reshape(128, TOT * RW)
        )
        tc_k = np.ascontiguousarray(Tp[sl].reshape(TOT, 128).T)
        rssw_k = np.ascontiguousarray(
            RSp[sl].reshape(TOT, 128, 2).transpose(1, 0, 2).reshape(128, TOT * 2)
        )
        in_maps.append(
            {
                "xt": xt_k,
                "tcols": tc_k,
                "rsswcols": rssw_k,
                "iota": iota_arr,
            }
        )
    return in_maps, wcorr


def _combine(results, wcorr):
    S = np.zeros((CSLOTS, D), dtype=np.float64)
    T = np.zeros((CSLOTS, D), dtype=np.float64)
    n = np.zeros(CSLOTS, dtype=np.float64)
    W = np.zeros(CSLOTS, dtype=np.float64)
    A = 0.0
    for k in range(NCORES):
        r = results[k]
        ost = np.asarray(r["o_st"], dtype=np.float64)
        A += float(np.asarray(r["o_stats"], dtype=np.float64).sum())
        for b in range(NBUCK):
            c0 = 128 * k + BW * b
            blk = ost[:, 130 * b : 130 * (b + 1)]
            S[c0 : c0 + BW] = blk[0:BW, 0:D]
            T[c0 : c0 + BW] = blk[BW : 2 * BW, 0:D]
            n[c0 : c0 + BW] = blk[0:BW, D]
            W[c0 : c0 + BW] = blk[0:BW, D + 1]

    A -= wcorr
    n_int = np.round(n)
    means = S / np.maximum(n_int, 1.0)[:, None]
    Wsum = W.sum()
    total = A - 2.0 * float((means * T).sum()) + float(
        (W * (means * means).sum(axis=1)).sum()
    )
    return np.float32(total / Wsum)


def kernel(inputs, targets, weights, num_classes):
    from concourse.bass_utils import run_bass_kernel_spmd

    x = np.asarray(inputs, dtype=np.float32)
    t = np.asarray(targets).astype(np.int64)
    w = np.asarray(weights, dtype=np.float32)
    assert int(num_classes) == C, f"compiled for {C} classes, got {num_classes}"
    assert x.shape == (N, D) and t.shape == (N,) and w.shape == (N,)

    in_maps, wcorr = _prepare_inputs(x, t, w)
    nc = _get_nc()
    res = run_bass_kernel_spmd(nc, in_maps, list(range(NCORES)))
    return _combine(res.results, wcorr)


if __name__ == "__main__":
    rng = np.random.default_rng(0)
    x = rng.standard_normal((N, D)).astype(np.float32)
    t = rng.integers(0, C, N).astype(np.int64)
    w = rng.random(N).astype(np.float32)
    out = kernel(x, t, w, C)
    print("kernel output:", out)
